# revision 91
# baseline (speedup 1.0000x reference)
"""DiffPoolEncoder Trainium2 kernel.

Sharding: data parallel by graph. 8 cores x 4 graphs (512 nodes each).
Host preprocessing is index-only (edge-list scatter + integer degree
counts): it ships the dense degree-scaled A^T blocks (bf16; entries
count/deg are exact in bf16) and the clamped-degree table directly, so
the device spends no time building the aggregation matrix.

Schedule: the assignment path (a1,a2,a3,logits,softmax) runs first, the
GC path (h1..h3) second with S^T h_L pooled contributions fused in as
each layer completes - no DRAM spills at all. The a-path activations are
bf16 and the a3/logits GEMMs run as fp8e4m3 DoubleRow matmuls (2 K-rows
per partition, 0.5 cyc/col); the h-path (readout-critical) stays f32r.
Weight/parameter DMAs are staggered to overlap compute windows.
"""

import sys

for _p in ("/opt/trn_rl_repo",):
    if _p not in sys.path:
        sys.path.append(_p)

import numpy as np
import ml_dtypes
from contextlib import ExitStack

import concourse.bass as bass
import concourse.mybir as mybir
import concourse.tile as tile
from concourse import bacc
from concourse.bass_utils import run_bass_kernel_spmd

F32 = mybir.dt.float32
F32R = mybir.dt.float32r
BF16 = mybir.dt.bfloat16
FP8 = mybir.dt.float8e4
AF = mybir.ActivationFunctionType
ALU = mybir.AluOpType
AX = mybir.AxisListType
DR = mybir.MatmulPerfMode.DoubleRow

NCORES = 8
B = 32
NPG = 512
G = 4            # graphs per core
T = 16           # node tiles per core (4 per graph)
NLOC = 2048      # nodes per core
K = 64           # clusters per graph
IN = 128
HID = 256
PWS = 64.0       # fp8 packing scale for pW (undone at the logits evac)

# bcol column layout (each 128-chunk of a bias vector is one column)
BC_B1, BC_B2, BC_B3 = 0, 2, 4
BC_AB1, BC_AB2 = 6, 8
BC_AB3 = 10          # 16 cols
BC_QB1, BC_QB2, BC_QB3 = 26, 28, 30
BC_MB1, BC_MB2 = 32, 34
BC_PB = 35           # 4 cols, per-graph pb slice on partitions 0:64
BC_N = 39

# rows2 [65, 1024] f32r: rows at matmul base partitions {0, 64};
# ones[0:512] replicated at each used partition (matmul needs equal bases).
R_QB1 = (0, 512)
R_QB2, R_QB3 = (64, 512), (64, 768)
ROWS_W = 1024


def build_module():
    nc = bacc.Bacc("TRN2", target_bir_lowering=False)

    # ---------------- DRAM I/O ----------------
    featT_d = nc.dram_tensor("featT", [128, NLOC], F32R, kind="ExternalInput")
    featnm_d = nc.dram_tensor("feat_nm", [128, T * IN], BF16, kind="ExternalInput")
    at_d = nc.dram_tensor("at_sc", [128, T * NPG], BF16, kind="ExternalInput")
    degcl_d = nc.dram_tensor("degcl", [128, T], F32, kind="ExternalInput")
    bcol_d = nc.dram_tensor("bcol", [128, BC_N], F32, kind="ExternalInput")
    rows_d = nc.dram_tensor("rows2", [65, ROWS_W], F32R, kind="ExternalInput")
    identb_d = nc.dram_tensor("identb", [128, 128], BF16, kind="ExternalInput")
    identr_d = nc.dram_tensor("identr", [128, 128], F32R, kind="ExternalInput")
    aW3q_d = nc.dram_tensor("aW3q", [128, 2 * 16 * 256], FP8, kind="ExternalInput")
    pWq_d = nc.dram_tensor("pWq", [128, 10 * 1024], FP8, kind="ExternalInput")
    atq_d = nc.dram_tensor("at_q", [128, T * NPG], FP8, kind="ExternalInput")
    aW2q_d = nc.dram_tensor("aW2q", [128, 2 * 512], FP8, kind="ExternalInput")
    w_d = {}
    for name, fi, fo, dt in [
        ("W1", 256, 256, F32R), ("W2", 512, 256, F32R), ("W3", 512, 256, F32R),
        ("aW1", 256, 256, F32R),
        ("qW1", 1536, 256, BF16), ("qW2", 512, 256, BF16),
        ("qW3", 512, 256, BF16), ("mW1", 1536, 256, F32R), ("mW2", 256, 10, F32R),
    ]:
        w_d[name] = nc.dram_tensor(name, [fi, fo], dt, kind="ExternalInput")
    yp_d = nc.dram_tensor("yp", [10, G], F32, kind="ExternalOutput")

    with tile.TileContext(nc) as tc, ExitStack() as ex, \
            nc.allow_low_precision(reason="tf32/bf16/fp8 matmuls; fp32 PSUM accum"):
        persist = ex.enter_context(tc.tile_pool(name="persist", bufs=1))
        # PSUM: 8 banks. One tag per pool so slot count == bank count.
        ps_p = ex.enter_context(tc.tile_pool(name="psP", bufs=4, space="PSUM"))
        lg_p = ex.enter_context(tc.tile_pool(name="psL", bufs=1, space="PSUM"))
        pm_p = ex.enter_context(tc.tile_pool(name="psM", bufs=2, space="PSUM"))
        pl_p = ex.enter_context(tc.tile_pool(name="psS", bufs=1, space="PSUM"))

        uid = [0]

        def _nm(pfx):
            uid[0] += 1
            return f"{pfx}{uid[0]}"

        def ps_big(dt=F32, w=512):
            return ps_p.tile([128, w], dt, tag="ps", name=_nm("ps"))

        def ps_med(p, f, dt=F32):
            return pm_p.tile([p, f], dt, tag="pm", name=_nm("pm"))

        def ps_sml(p, f, dt=F32):
            return pl_p.tile([p, f], dt, tag="pl", name=_nm("pl"))

        def wload(pool, name, fi, fo, dt=F32R):
            kk = fi // 128
            sb = pool.tile([128, kk * fo], dt, tag=name, name=name)
            nc.sync.dma_start(
                sb[:].rearrange("p (k f) -> p k f", k=kk, f=fo),
                w_d[name][:, :].rearrange("(k p) f -> p k f", p=128),
            )
            return sb

        # PSUM-evacuation spread across Activation and DVE engines; Act gets
        # 3 of 5 (DVE also carries the reduces/softmax work)
        ev = [0]

        def evac_copy(dst, src):
            ev[0] += 1
            if ev[0] % 2:
                nc.vector.tensor_copy(dst, src)
            else:
                nc.scalar.copy(dst, src)

        def evac_act(dst, src, relu, bccol):
            ev[0] += 1
            if ev[0] % 2 == 0:
                nc.scalar.activation(dst, src, AF.Relu if relu else AF.Identity,
                                     bias=bcol[:, bccol : bccol + 1])
            elif relu:
                nc.vector.tensor_scalar(dst, src, bcol[:, bccol : bccol + 1],
                                        0.0, op0=ALU.add, op1=ALU.max)
            else:
                nc.vector.tensor_scalar(dst, src, bcol[:, bccol : bccol + 1],
                                        None, op0=ALU.add)

        # ---------- persistent small tensors ----------
        identb = persist.tile([128, 128], BF16)
        identr = persist.tile([128, 128], F32R)
        rows2 = persist.tile([65, ROWS_W], F32R)
        bcol = persist.tile([128, BC_N], F32)
        degcl = persist.tile([128, T], F32)
        S_nm = persist.tile([128, T * K], F32R)   # exp scratch
        S_nb = persist.tile([128, T * K], BF16)
        out_fm = persist.tile([128, 12 * G], F32R)  # readout maxes, col=ch*G+g
        nmax = persist.tile([128, 2], F32)
        sumx = persist.tile([128, 2], F32)
        y_sb = persist.tile([128, 2 * G], F32R)
        z_sb = persist.tile([10, G], F32)


        def ones_at(p, n):
            return rows2[p : p + 1, 0:n]

        def rrow(ro, n):
            p, off = ro
            return rows2[p : p + 1, off : off + n]

        # scaled A^T (bf16, host-built) lives to the end
        at_p = ex.enter_context(tc.tile_pool(name="atp", bufs=1))
        AT = at_p.tile([128, T * NPG], BF16)

        # feat (both layouts) + aggfeat + h1/h3 slot-shared tensors.
        # h1f/h1n die early in phase B exactly where h3f/h3n are born, so the
        # pairs share slots (tags xh/xn) in this whole-kernel pool.
        fa_p = ex.enter_context(tc.tile_pool(name="fap", bufs=1))
        featnm = fa_p.tile([128, T * IN], BF16, tag="featnm")
        featT = fa_p.tile([128, NLOC], F32R, tag="featT")
        aggfeat = fa_p.tile([128, NLOC], F32R, tag="aggf")
        # DMA order = criticality: graph-0 feat + A^T feed the first
        # aggregation almost immediately; everything else follows.
        for g in range(G):
            nc.sync.dma_start(featnm[:, g * 512 : (g + 1) * 512],
                              featnm_d[:, g * 512 : (g + 1) * 512])
            nc.sync.dma_start(AT[:, g * 2048 : (g + 1) * 2048],
                              at_d[:, g * 2048 : (g + 1) * 2048])
            if g == 0:
                nc.sync.dma_start(identb[:], identb_d[:])
                nc.sync.dma_start(identr[:], identr_d[:])
                nc.sync.dma_start(bcol[:], bcol_d[:])

        # ---------- a-phase pools (LIFO: close-first at top) ----------
        exQ = ExitStack()    # closes after softmax
        q_p = exQ.enter_context(tc.tile_pool(name="qp", bufs=1))
        exW = ExitStack()    # closes after h1
        wA_p = exW.enter_context(tc.tile_pool(name="wA", bufs=1))
        exA2 = ExitStack()   # closes after agga2
        aNm_p = exA2.enter_context(tc.tile_pool(name="aNm", bufs=2))
        a2_p = exA2.enter_context(tc.tile_pool(name="a2p", bufs=1))
        exA1 = ExitStack()   # closes after a1 transposes + a1q
        a1_p = exA1.enter_context(tc.tile_pool(name="a1p", bufs=1))

        # fp8 operand tiles for the a3/logits GEMMs + fp8 A^T for a-path aggs
        a1q = q_p.tile([128, G * 1024], FP8, tag="a1q")
        a2q = q_p.tile([128, G * 1024], FP8, tag="a2q")
        agga1q = q_p.tile([128, G * 1024], FP8, tag="ag1q")
        agga2q = q_p.tile([128, G * 1024], FP8, tag="ag2q")
        aW2q = q_p.tile([128, 2 * 512], FP8, tag="aW2q")
        aW3q = q_p.tile([128, 2 * 16 * 256], FP8, tag="aW3q")
        pWq = q_p.tile([128, 10 * 1024], FP8, tag="pWq")
        a3q = q_p.tile([128, 16 * 512], FP8, tag="a3q")
        ATq = q_p.tile([128, T * NPG], FP8, tag="ATq")
        lgs_nm = q_p.tile([128, T * K], F32, tag="lgs")

        W1 = wload(wA_p, "W1", 256, 256)
        aW1 = wload(wA_p, "aW1", 256, 256)
        nc.sync.dma_start(featT[:, 0:1024], featT_d[:, 0:1024])
        nc.sync.dma_start(featT[:, 1024:2048], featT_d[:, 1024:2048])
        nc.sync.dma_start(aW2q[:], aW2q_d[:])
        nc.sync.dma_start(ATq[:], atq_d[:])
        nc.sync.dma_start(aW3q[:], aW3q_d[:])
        nc.sync.dma_start(pWq[:], pWq_d[:])
        W2 = wload(fa_p, "W2", 512, 256)
        aggh1 = fa_p.tile([128, 2 * NLOC], F32R, tag="agh1", name="aggh1")

        # ---------- emit helpers ----------
        def emit_agg(x_nm, D, dst_of, graphs=tuple(range(G))):
            """dst(g, ch) slice [128,512] (feature-major) = sum_s x*AT."""
            for g in graphs:
                for ch in range(D // 128):
                    ps = ps_big()
                    for st in range(4):
                        t = g * 4 + st
                        nc.tensor.matmul(
                            ps[:],
                            lhsT=x_nm[:, t * D + ch * 128 : t * D + ch * 128 + 128],
                            rhs=AT[:, t * NPG : (t + 1) * NPG],
                            start=(st == 0), stop=(st == 3))
                    evac_copy(dst_of(g, ch), ps[:])

        def emit_lin_fm(x_fm, a_fm, Din, Wsb, bccol, relu, out_t):
            nk = Din // 128
            for co in range(2):
                for nb in range(4):
                    ps = ps_big()
                    ki = 0
                    for src in (x_fm, a_fm):
                        for ci in range(nk):
                            nc.tensor.matmul(
                                ps[:],
                                lhsT=Wsb[:, ki * 256 + co * 128 : ki * 256 + co * 128 + 128],
                                rhs=src[:, ci * NLOC + nb * 512 : ci * NLOC + (nb + 1) * 512],
                                start=(ki == 0), stop=(ki == 2 * nk - 1))
                            ki += 1
                    evac_act(out_t[:, co * NLOC + nb * 512 : co * NLOC + (nb + 1) * 512],
                             ps[:], relu, bccol + co)

        def emit_nm_T_f32(x_fm, out_nm):
            # node-major via PE transposes; 4 blocks share one PSUM bank.
            # out_nm is bf16 (cast at the evac) so aggregation matmuls can
            # pair it with the bf16 A^T.
            for t2 in range(0, T, 2):
                tp = ps_big(F32R)
                for i, (t, ch) in enumerate(
                        ((t2, 0), (t2, 1), (t2 + 1, 0), (t2 + 1, 1))):
                    nc.tensor.matmul(
                        tp[:, i * 128 : (i + 1) * 128],
                        lhsT=x_fm[:, ch * NLOC + t * 128 : (ch * NLOC + t * 128) + 128],
                        rhs=identr[:], is_transpose=True,
                        start=True, stop=True, skip_group_check=True)
                evac_copy(out_nm[:, t2 * HID : (t2 + 2) * HID], tp[:])

        def emit_nm_T_bf16(x_fm, out_nm):
            # bf16 transposes: 8 blocks per PSUM bank (1024 bf16 cols).
            for t4 in range(0, T, 4):
                tp = ps_big(BF16, 1024)
                for i in range(8):
                    t, ch = t4 + i // 2, i % 2
                    nc.tensor.matmul(
                        tp[:, i * 128 : (i + 1) * 128],
                        lhsT=x_fm[:, ch * NLOC + t * 128 : (ch * NLOC + t * 128) + 128],
                        rhs=identb[:], is_transpose=True,
                        start=True, stop=True, skip_group_check=True)
                evac_copy(out_nm[:, t4 * HID : (t4 + 4) * HID], tp[:])

        def emit_out1(x_fm, ch0):
            for ci in range(2):
                for g in range(G):
                    nc.vector.tensor_reduce(
                        out_fm[:, (ch0 + ci) * G + g : (ch0 + ci) * G + g + 1],
                        x_fm[:, ci * NLOC + g * NPG : ci * NLOC + (g + 1) * NPG],
                        axis=AX.X, op=ALU.max)

        def emit_agg_dr(x_nq, dst_of):
            # a-path aggregation, fp8 DoubleRow: two source chunks/partition
            for g in range(G):
                for ch in range(2):
                    ps = ps_big()
                    for tp2 in range(2):
                        t = g * 4 + tp2 * 2
                        nc.tensor.matmul(
                            ps[:],
                            lhsT=x_nq[:, t * HID : (t + 2) * HID].rearrange(
                                "p (two d) -> p two d",
                                two=2)[:, :, ch * 128 : (ch + 1) * 128],
                            rhs=ATq[:, t * NPG : (t + 2) * NPG].rearrange(
                                "p (two n) -> p two n", two=2),
                            start=(tp2 == 0), stop=(tp2 == 1), perf_mode=DR)
                    evac_copy(dst_of(g, ch), ps[:])

        def to_q(x_fm, dst):
            # bf16 fm [p, ch*NLOC + g*512 + n] -> fp8 [p, g*1024 + ch*512 + n]
            for g in range(G):
                for ch in range(2):
                    nc.gpsimd.tensor_copy(
                        dst[:, g * 1024 + ch * 512 : g * 1024 + (ch + 1) * 512],
                        x_fm[:, ch * NLOC + g * NPG : ch * NLOC + (g + 1) * NPG])

        def dr_rhs(tile_, g):
            # [p][2][512] from [p, g*1024 + h*512 + n] layout
            return tile_[:, g * 1024 : (g + 1) * 1024].rearrange(
                "p (h x) -> p h x", h=2)

        # ================= phase A: assignment path =================
        emit_agg(featnm, IN,
                 lambda g, ch: aggfeat[:, g * NPG : (g + 1) * NPG])

        a1f = a1_p.tile([128, 2 * NLOC], BF16, tag="a1f", name="a1f")
        a1n = aNm_p.tile([128, T * HID], FP8, tag="aNm", name="a1n")
        emit_lin_fm(featT, aggfeat, 128, aW1, BC_AB1, True, a1f)
        emit_nm_T_bf16(a1f, a1n)
        to_q(a1f, a1q)
        exA1.close()

        emit_agg_dr(a1n,
                    lambda g, ch: agga1q[:, g * 1024 + ch * 512 : g * 1024 + (ch + 1) * 512])

        # a2 linear in fp8 DoubleRow (rhs = a1q / agga1q, per graph)
        a2f = a2_p.tile([128, 2 * NLOC], BF16, tag="a2f", name="a2f")
        a2n = aNm_p.tile([128, T * HID], FP8, tag="aNm", name="a2n")
        for g in range(G):
            for co in range(2):
                ps = ps_big()
                for s, src in enumerate((a1q, agga1q)):
                    nc.tensor.matmul(
                        ps[:],
                        lhsT=aW2q[:, s * 512 + co * 256 : s * 512 + (co + 1) * 256]
                        .rearrange("p (h m) -> p h m", h=2),
                        rhs=dr_rhs(src, g),
                        start=(s == 0), stop=(s == 1), perf_mode=DR)
                evac_act(a2f[:, co * NLOC + g * NPG : co * NLOC + (g + 1) * NPG],
                         ps[:], True, BC_AB2 + co)

        # h1 interleaves here: independent work keeps PE fed during a2 evacs
        h1f = fa_p.tile([128, 2 * NLOC], F32R, tag="xh", name="h1f")
        h1n = fa_p.tile([128, T * HID], BF16, tag="xn", name="h1n")
        emit_lin_fm(featT, aggfeat, 128, W1, BC_B1, True, h1f)
        emit_out1(h1f, 0)

        emit_nm_T_bf16(a2f, a2n)
        to_q(a2f, a2q)
        emit_nm_T_f32(h1f, h1n)

        emit_agg_dr(a2n,
                    lambda g, ch: agga2q[:, g * 1024 + ch * 512 : g * 1024 + (ch + 1) * 512])
        exA2.close()
        exW.close()

        # ---------- a3 + logits, fp8 DoubleRow, streamed per graph ----------
        for g in range(G):
            for co in range(16):
                ps3 = ps_big()
                for s, src in enumerate((a2q, agga2q)):
                    lo = s * 4096 + co * 256
                    nc.tensor.matmul(
                        ps3[:],
                        lhsT=aW3q[:, lo : lo + 256].rearrange(
                            "p (h m) -> p h m", h=2),
                        rhs=dr_rhs(src, g),
                        start=(s == 0), stop=(s == 1), perf_mode=DR)
                evac_act(a3q[:, co * 512 : (co + 1) * 512], ps3[:], True, BC_AB3 + co)
            lps = lg_p.tile([128, 512], F32, tag="lg", name=_nm("lg"))
            for q in range(10):
                lhsT = pWq[:, q * 1024 + g * 256 : q * 1024 + (g + 1) * 256].rearrange(
                    "p (h m) -> p h m", h=2)
                if q < 2:
                    rhs = dr_rhs((a1q, a2q)[q], g)
                else:
                    rhs = a3q[:, (q - 2) * 1024 : (q - 1) * 1024].rearrange(
                        "p (c x) -> p c x", c=2)
                nc.tensor.matmul(
                    lps[:], lhsT=lhsT, rhs=rhs,
                    start=(q == 0), stop=(q == 9), perf_mode=DR)
            lgf = q_p.tile([64, 512], F32R, tag="lgf", name=_nm("lgf"), bufs=1)
            nc.scalar.activation(lgf[:], lps[0:64, :], AF.Identity,
                                 scale=1.0 / PWS,
                                 bias=bcol[0:64, BC_PB + g : BC_PB + g + 1])
            for j in range(4):  # transpose to node-major
                t = g * 4 + j
                tps = ps_med(128, 64, F32R)
                nc.tensor.matmul(
                    tps[:], lhsT=lgf[0:64, j * 128 : (j + 1) * 128],
                    rhs=identr[0:64, 0:64], is_transpose=True,
                    start=True, stop=True, skip_group_check=True)
                evac_copy(lgs_nm[:, t * K : (t + 1) * K], tps[:])
            if g < 2:
                # fill PE gaps of the evac-bound a3 window with the h1 agg
                emit_agg(h1n, HID,
                         lambda gg, ch: aggh1[:, ch * NLOC + gg * NPG : ch * NLOC + (gg + 1) * NPG],
                         graphs=(2 * g, 2 * g + 1))

        # masked softmax == per-graph softmax over K columns
        for t in range(T):
            bb = t % 2
            nc.vector.tensor_reduce(nmax[:, bb : bb + 1], lgs_nm[:, t * K : (t + 1) * K],
                                    axis=AX.X, op=ALU.max, negate=True)
            nc.scalar.activation(S_nm[:, t * K : (t + 1) * K],
                                 lgs_nm[:, t * K : (t + 1) * K], AF.Exp,
                                 bias=nmax[:, bb : bb + 1],
                                 accum_out=sumx[:, bb : bb + 1])
            nc.vector.reciprocal(sumx[:, bb : bb + 1], sumx[:, bb : bb + 1])
            nc.vector.tensor_scalar(S_nb[:, t * K : (t + 1) * K],
                                    S_nm[:, t * K : (t + 1) * K],
                                    sumx[:, bb : bb + 1], None, op0=ALU.mult)

        # h2 keeps PE busy while DVE/Act run the softmax chain
        h2f = fa_p.tile([128, 2 * NLOC], F32R, tag="h2f", name="h2f")
        h2n = fa_p.tile([128, T * HID], BF16, tag="h2n", name="h2n")
        emit_lin_fm(h1f, aggh1, 256, W2, BC_B2, True, h2f)
        emit_out1(h2f, 2)
        emit_nm_T_f32(h2f, h2n)
        exQ.close()

        # ================= phase B: GC path + pooling =================
        late_p = ex.enter_context(tc.tile_pool(name="late", bufs=1))
        # weight DMAs first so they overlap the AS/adj/h2 compute
        nc.sync.dma_start(degcl[:], degcl_d[:])
        nc.sync.dma_start(rows2[:], rows_d[:])
        W3 = wload(late_p, "W3", 512, 256)
        qW1 = wload(late_p, "qW1", 1536, 256, BF16)
        qW2 = wload(late_p, "qW2", 512, 256, BF16)
        qW3 = wload(late_p, "qW3", 512, 256, BF16)
        mW1 = wload(late_p, "mW1", 1536, 256)
        mW2 = wload(late_p, "mW2", 256, 10)
        hp_nm = late_p.tile([128, 2 * 768], BF16, tag="hpn", name="hp_nm")
        hp_fm = late_p.tile([128, 6 * 256], BF16, tag="hpf", name="hp_fm")
        AS_nb = late_p.tile([128, T * K], BF16, tag="AS", name="AS_nb")

        def emit_hp(x_nm, L):
            # h_pool contribution of layer L: S^T x per graph, pair-stacked
            for h in range(2):
                for gs in range(2):
                    g = h * 2 + gs
                    ps = ps_med(64, 256)
                    for j in range(4):
                        t = g * 4 + j
                        nc.tensor.matmul(
                            ps[:],
                            lhsT=S_nb[:, t * K : (t + 1) * K],
                            rhs=x_nm[:, t * HID : (t + 1) * HID],
                            start=(j == 0), stop=(j == 3))
                    dst = hp_nm[gs * 64 : gs * 64 + 64,
                                h * 768 + L * 256 : h * 768 + (L + 1) * 256]
                    if gs == 0:
                        evac_copy(dst, ps[:])
                    else:
                        sh = late_p.tile([64, 256], BF16, tag="hpsh",
                                         name=_nm("hpsh"), bufs=2)
                        evac_copy(sh[:], ps[:])
                        nc.sync.dma_start(dst, sh[:])

        emit_hp(h1n, 0)

        # AS = A @ S (scaled-AT product un-scaled by clamped deg; exact)
        for t in range(T):
            g, j = t // 4, t % 4
            ps = ps_sml(128, K)
            for st in range(4):
                nc.tensor.matmul(
                    ps[:],
                    lhsT=AT[:, (g * 4 + st) * NPG + j * 128 : (g * 4 + st) * NPG + (j + 1) * 128],
                    rhs=S_nb[:, (g * 4 + st) * K : (g * 4 + st + 1) * K],
                    start=(st == 0), stop=(st == 3))
            ev[0] += 1
            if ev[0] % 2:
                nc.vector.tensor_scalar(AS_nb[:, t * K : (t + 1) * K], ps[:],
                                        degcl[:, t : t + 1], None, op0=ALU.mult)
            else:
                nc.scalar.activation(AS_nb[:, t * K : (t + 1) * K], ps[:],
                                     AF.Identity, scale=degcl[:, t : t + 1])

        emit_hp(h2n, 1)

        # adj = S^T (A S), pair-stacked; row-normalized
        adjg = late_p.tile([128, 2 * K], F32R, tag="adjg", name="adjg")
        rsum = late_p.tile([128, 2], F32, tag="rsum", name="rsum")
        adjT = late_p.tile([128, 2 * 128], BF16, tag="adjT", name="adjT")
        nc.vector.memset(adjT[:], 0.0)
        for h in range(2):
            for gs in range(2):
                g = h * 2 + gs
                ps = ps_sml(64, K)
                for j in range(4):
                    t = g * 4 + j
                    nc.tensor.matmul(ps[:],
                                     lhsT=S_nb[:, t * K : (t + 1) * K],
                                     rhs=AS_nb[:, t * K : (t + 1) * K],
                                     start=(j == 0), stop=(j == 3))
                dst = adjg[gs * 64 : gs * 64 + 64, h * K : (h + 1) * K]
                if gs == 0:
                    evac_copy(dst, ps[:])
                else:
                    sh = late_p.tile([64, K], F32R, tag="adsh",
                                     name=_nm("adsh"), bufs=2)
                    evac_copy(sh[:], ps[:])
                    nc.sync.dma_start(dst, sh[:])
            nc.vector.tensor_reduce(rsum[:, h : h + 1], adjg[:, h * K : (h + 1) * K],
                                    axis=AX.X, op=ALU.add)
            nc.vector.tensor_scalar(rsum[:, h : h + 1], rsum[:, h : h + 1],
                                    1e-9, None, op0=ALU.add)
            nc.vector.reciprocal(rsum[:, h : h + 1], rsum[:, h : h + 1])
            nc.vector.tensor_scalar(adjg[:, h * K : (h + 1) * K],
                                    adjg[:, h * K : (h + 1) * K],
                                    rsum[:, h : h + 1], None, op0=ALU.mult)
            # transpose each graph's [64,64] block onto the block diagonal
            for gs in range(2):
                tp = ps_sml(128, K, F32R)
                nc.tensor.matmul(
                    tp[0:64, :],
                    lhsT=adjg[gs * 64 : gs * 64 + 64, h * K : (h + 1) * K],
                    rhs=identr[gs * 64 : gs * 64 + 64, gs * 64 : gs * 64 + 64],
                    is_transpose=True, start=True, stop=True,
                    skip_group_check=True)
                if gs == 0:
                    nc.vector.tensor_copy(adjT[0:64, h * 128 : h * 128 + 64],
                                          tp[0:64, :])
                else:
                    sb = late_p.tile([64, K], BF16, tag="adjsh", name=_nm("adjsh"),
                                     bufs=2)
                    nc.vector.tensor_copy(sb[:], tp[0:64, :])
                    nc.sync.dma_start(
                        adjT[64:128, h * 128 + 64 : h * 128 + 128], sb[:])

        aggh2 = late_p.tile([128, 2 * NLOC], F32R, tag="hag", name="aggh2")
        emit_agg(h2n, HID,
                 lambda g, ch: aggh2[:, ch * NLOC + g * NPG : ch * NLOC + (g + 1) * NPG])

        h3f = fa_p.tile([128, 2 * NLOC], F32R, tag="xh", name="h3f")
        h3n = fa_p.tile([128, T * HID], BF16, tag="xn", name="h3n")
        emit_lin_fm(h2f, aggh2, 256, W3, BC_B3, False, h3f)
        emit_out1(h3f, 4)
        emit_nm_T_f32(h3f, h3n)
        emit_hp(h3n, 2)

        # hp_fm via transposes of the pair tiles
        for h in range(2):
            for ch in range(6):
                tp = ps_med(128, 128, BF16)
                nc.tensor.matmul(
                    tp[:], lhsT=hp_nm[:, h * 768 + ch * 128 : h * 768 + (ch + 1) * 128],
                    rhs=identb[:], is_transpose=True,
                    start=True, stop=True, skip_group_check=True)
                evac_copy(hp_fm[:, ch * 256 + h * 128 : ch * 256 + (h + 1) * 128],
                          tp[:])

        # ---------- pooled sage stack (pair-batched) ----------
        hn1_fm = late_p.tile([128, 6 * 256], BF16, tag="hn1", name="hn1_fm")
        p1_nm = late_p.tile([128, 2 * 256], BF16, tag="p1n", name="p1_nm")
        p1_fm = late_p.tile([128, 2 * 256], BF16, tag="p1f", name="p1_fm")
        hn2_fm = late_p.tile([128, 2 * 256], BF16, tag="hn2", name="hn2_fm")
        p2_nm = late_p.tile([128, 2 * 256], BF16, tag="p2n", name="p2_nm")
        p2_fm = late_p.tile([128, 2 * 256], BF16, tag="p2f", name="p2_fm")
        hn3_fm = late_p.tile([128, 2 * 256], BF16, tag="hn3", name="hn3_fm")
        p3_fm = late_p.tile([128, 2 * 256], BF16, tag="p3f", name="p3_fm")

        def pool_hn(x_nm, xw, out_t):
            # out[d, u-pair] = sum_{v-pair} x_nm[v, d] * adjT_bd[v, u]
            for h in range(2):
                for ch in range(xw // 128):
                    tp = ps_sml(128, 128)
                    nc.tensor.matmul(
                        tp[:],
                        lhsT=x_nm[:, h * xw + ch * 128 : h * xw + (ch + 1) * 128],
                        rhs=adjT[:, h * 128 : (h + 1) * 128],
                        start=True, stop=True)
                    evac_copy(out_t[:, ch * 256 + h * 128 : ch * 256 + (h + 1) * 128],
                              tp[:])

        def pool_lin(xf, hf, Din, Wsb, bccol, rbias, relu, outf, outn):
            nch = Din // 256
            for co in range(2):
                ps = ps_med(128, 256)
                ki = 0
                for src in (xf, hf):
                    for ch in range(nch):
                        nc.tensor.matmul(
                            ps[:],
                            lhsT=Wsb[:, ki * 256 + co * 128 : ki * 256 + co * 128 + 128],
                            rhs=src[:, ch * 256 : (ch + 1) * 256],
                            start=(ki == 0), stop=(ki == 2 * nch - 1))
                        ki += 1
                evac_act(outf[:, co * 256 : (co + 1) * 256], ps[:], relu, bccol + co)
            if outn is not None:
                for h in range(2):
                    ps = ps_med(128, 256)
                    ki = 0
                    for src in (xf, hf):
                        for ch in range(nch):
                            nc.tensor.matmul(
                                ps[:],
                                lhsT=src[:, ch * 256 + h * 128 : ch * 256 + (h + 1) * 128],
                                rhs=Wsb[:, ki * 256 : (ki + 1) * 256],
                                start=(ki == 0), stop=False)
                            ki += 1
                    nc.tensor.matmul(ps[:], lhsT=ones_at(rbias[0], 128),
                                     rhs=rrow(rbias, 256),
                                     start=False, stop=True)
                    nc.vector.tensor_scalar(outn[:, h * 256 : (h + 1) * 256], ps[:],
                                            0.0, None, op0=ALU.max)

        def p_readout(L, pf):
            for co in range(2):
                for g in range(G):
                    nc.vector.tensor_reduce(
                        out_fm[:, (6 + L * 2 + co) * G + g : (6 + L * 2 + co) * G + g + 1],
                        pf[:, co * 256 + g * K : co * 256 + (g + 1) * K],
                        axis=AX.X, op=ALU.max)

        pool_hn(hp_nm, 768, hn1_fm)
        pool_lin(hp_fm, hn1_fm, 1536, qW1, BC_QB1, R_QB1, True, p1_fm, p1_nm)
        p_readout(0, p1_fm)
        pool_hn(p1_nm, 256, hn2_fm)
        pool_lin(p1_fm, hn2_fm, 512, qW2, BC_QB2, R_QB2, True, p2_fm, p2_nm)
        p_readout(1, p2_fm)
        pool_hn(p2_nm, 256, hn3_fm)
        pool_lin(p2_fm, hn3_fm, 512, qW3, BC_QB3, R_QB3, False, p3_fm, None)
        p_readout(2, p3_fm)

        # ---------- final MLP ----------
        for co in range(2):
            ps = ps_sml(128, G)
            for k in range(12):
                nc.tensor.matmul(
                    ps[:], lhsT=mW1[:, k * 256 + co * 128 : k * 256 + co * 128 + 128],
                    rhs=out_fm[:, k * G : (k + 1) * G],
                    start=(k == 0), stop=(k == 11))
            nc.scalar.activation(y_sb[:, co * G : (co + 1) * G], ps[:], AF.Identity,
                                 bias=bcol[:, BC_MB1 + co : BC_MB1 + co + 1])
        zps = ps_sml(10, G)
        for ci in range(2):
            nc.tensor.matmul(zps[:], lhsT=mW2[:, ci * 10 : (ci + 1) * 10],
                             rhs=y_sb[:, ci * G : (ci + 1) * G],
                             start=(ci == 0), stop=(ci == 1))
        nc.scalar.activation(z_sb[:], zps[:], AF.Identity,
                             bias=bcol[0:10, BC_MB2 : BC_MB2 + 1])
        nc.sync.dma_start(yp_d[:], z_sb[:])

    nc.compile()
    return nc


# ---------------------------------------------------------------------------
# host side
# ---------------------------------------------------------------------------

BF = ml_dtypes.bfloat16
F8 = ml_dtypes.float8_e4m3


def _pack_bcol(b):
    bc = np.zeros((128, BC_N), np.float32)
    for off, k in ((BC_B1, "b1"), (BC_B2, "b2"), (BC_B3, "b3"), (BC_AB1, "ab1"),
                   (BC_AB2, "ab2"), (BC_AB3, "ab3"), (BC_QB1, "qb1"),
                   (BC_QB2, "qb2"), (BC_QB3, "qb3"), (BC_MB1, "mb1")):
        v = np.asarray(b[k], np.float32)
        bc[:, off : off + v.size // 128] = v.reshape(-1, 128).T
    mb2 = np.asarray(b["mb2"], np.float32)
    bc[: mb2.size, BC_MB2] = mb2
    return bc


def tf32_round(v):
    u = np.ascontiguousarray(np.asarray(v, np.float32)).view(np.uint32).copy()
    u &= np.uint32(0xFFFFE000)
    return u.view(np.float32)


def _pack_rows(b):
    r = np.zeros((65, ROWS_W), np.float32)
    for p in (0, 64):
        r[p, 0:512] = 1.0
    for (p, off), k in ((R_QB1, "qb1"), (R_QB2, "qb2"), (R_QB3, "qb3")):
        r[p, off : off + 256] = b[k]
    return tf32_round(r)


def _dense_at(edge_src, edge_dst, core):
    """Dense degree-scaled A^T tiles + clamped-degree table for one core."""
    lo, hi = core * NLOC, (core + 1) * NLOC
    m = (edge_dst >= lo) & (edge_dst < hi)
    src = edge_src[m].astype(np.int64)
    dst = edge_dst[m].astype(np.int64)
    if not np.array_equal(src // NPG, dst // NPG):
        raise ValueError("cross-graph edges break graph-parallel sharding")
    sl = src - lo
    dl = dst - lo
    deg = np.bincount(dl, minlength=NLOC).astype(np.float64)
    degc = np.maximum(deg, 1.0)
    g = dl // NPG
    t_src = g * 4 + (sl % NPG) // 128
    p_src = sl % 128
    at = np.zeros((128, T * NPG), np.float64)
    np.add.at(at, (p_src, t_src * NPG + dl % NPG), 1.0 / degc[dl])
    degcl = degc.reshape(T, 128).T.astype(np.float32)
    return at.astype(BF), np.ascontiguousarray(degcl)


def _pack_aW2q(aW2):
    # [p, s*512 + co*256 + h*128 + m] = aW2[s*256+h*128+p, co*128+m]
    A = np.asarray(aW2, np.float32).reshape(2, 2, 128, 2, 128)
    return np.ascontiguousarray(
        A.transpose(2, 0, 3, 1, 4).reshape(128, 1024).astype(F8))


def _pack_aW3q(aW3):
    # [p, s*4096 + co*256 + h*128 + m] = aW3[s*256+h*128+p, co*128+m]
    A = np.asarray(aW3, np.float32).reshape(2, 2, 128, 16, 128)
    return np.ascontiguousarray(
        A.transpose(2, 0, 3, 1, 4).reshape(128, 8192).astype(F8))


def _pack_pWq(pW_lc):
    # [p, q*1024 + g*256 + h*128 + k] = PWS * pW[q*256+h*128+p, g*64+k], k<64;
    # zero-padded to 128 output columns so DoubleRow outputs 128 partitions
    P = (np.asarray(pW_lc, np.float32) * PWS).reshape(10, 2, 128, 4, 64)
    Pp = np.zeros((10, 2, 128, 4, 128), np.float32)
    Pp[..., :64] = P
    return np.ascontiguousarray(
        Pp.transpose(2, 0, 3, 1, 4).reshape(128, 10240).astype(F8))


_CACHE = {}
TRACE = False


def prepare_in_maps(inputs):
    f32 = lambda x: np.ascontiguousarray(np.asarray(x, np.float32))
    feat = f32(inputs["feat"])
    edge_src = np.asarray(inputs["edge_src"])
    edge_dst = np.asarray(inputs["edge_dst"])
    W = {k: tf32_round(inputs[k]) for k in
         ("W1", "W2", "W3", "aW1", "pW", "mW1", "mW2")}
    Wb = {k: np.ascontiguousarray(np.asarray(inputs[k], np.float32).astype(BF))
          for k in ("qW1", "qW2", "qW3")}
    b = {k: f32(inputs[k]) for k in
         ("b1", "b2", "b3", "ab1", "ab2", "ab3", "pb", "qb1", "qb2", "qb3",
          "mb1", "mb2")}
    ident = np.eye(128, dtype=np.float32)
    bcol = _pack_bcol(b)
    aW3q = _pack_aW3q(inputs["aW3"])
    aW2q = _pack_aW2q(inputs["aW2"])

    in_maps = []
    for c in range(NCORES):
        fs = feat[c * NLOC : (c + 1) * NLOC]
        feat_nm = np.ascontiguousarray(
            fs.reshape(T, 128, IN).transpose(1, 0, 2).reshape(128, T * IN))
        featT = np.ascontiguousarray(fs.T)
        at_sc, degcl = _dense_at(edge_src, edge_dst, c)
        at_q = np.ascontiguousarray(
            np.asarray(at_sc, np.float32).astype(F8))
        pW_lc = np.ascontiguousarray(W["pW"][:, c * G * K : (c + 1) * G * K])
        pb_lc = np.ascontiguousarray(b["pb"][c * G * K : (c + 1) * G * K])
        bc = bcol.copy()
        for g in range(G):
            bc[0:64, BC_PB + g] = pb_lc[g * K : (g + 1) * K]
        in_maps.append({
            "featT": tf32_round(featT), "feat_nm": feat_nm.astype(BF),
            "at_sc": at_sc, "at_q": at_q, "degcl": degcl,
            "bcol": bc, "rows2": _pack_rows(b),
            "identb": ident.astype(BF), "identr": ident,
            "aW3q": aW3q, "aW2q": aW2q, "pWq": _pack_pWq(pW_lc),
            "W1": W["W1"], "W2": W["W2"], "W3": W["W3"],
            "aW1": W["aW1"],
            "qW1": Wb["qW1"], "qW2": Wb["qW2"], "qW3": Wb["qW3"],
            "mW1": W["mW1"], "mW2": W["mW2"],
        })
    return in_maps


def kernel(**inputs):
    if "nc" not in _CACHE:
        _CACHE["nc"] = build_module()
    nc = _CACHE["nc"]
    in_maps = prepare_in_maps(inputs)
    res = run_bass_kernel_spmd(nc, in_maps, core_ids=list(range(NCORES)),
                               trace=TRACE)
    _CACHE["last_res"] = res
    out = np.zeros((B, 10), np.float32)
    for c in range(NCORES):
        out[c * G : (c + 1) * G, :] = np.asarray(res.results[c]["yp"]).T
    return out
